# revision 11
# baseline (speedup 1.0000x reference)
"""Trainium2 Bass kernel for nn_AdaptiveMask: out = x * ring_mask(current_val).

x: [32, 8, 256, 256] f32.  mask: [256, 256] computed from the scalar
current_val (concentric-ring ramp, values in [0, 1]).

Strategy (memory-bound, pure elementwise):
  - Shard x along batch dim: 4 batches per core across 8 cores (data parallel).
  - Host precomputes the [256, 256] mask from current_val.
  - Tile = one [256, 256] image as an SBUF tile [128, 512]: partition p holds
    image rows 2p and 2p+1 (row-major contiguous), so the SBUF mask tile is
    exactly mask.reshape(128, 512) - 256 KiB, fully unique, no replication.
  - Per core: 32 image tiles. Loads on nc.sync (HWDGE ring 0), multiply on
    DVE in-place, stores on nc.scalar (HWDGE ring 1). Tile framework
    pipelines with a multi-buffer pool.
  - Per-core HBM traffic: 8 MiB in + 0.25 MiB mask + 8 MiB out.
"""

import sys

import numpy as np

for _p in ("/opt/trn_rl_repo",):
    if _p not in sys.path:
        sys.path.append(_p)

from concourse import bacc, bass, tile
from concourse.bass import mybir
from concourse.bass_utils import run_bass_kernel_spmd

N_CORES = 8
B, H, N = 32, 8, 256
MAX_SIZE = 256
RAMP_SIZE = 32

IMGS = (B // N_CORES) * H  # 32 images per core
TILE_P = 128
TILE_F = 512  # one [256, 256] image = [128, 512] f32 = 256 KiB

_cache = {}


def _build_program(bufs=8):
    nc = bacc.Bacc(None, target_bir_lowering=False)
    x_in = nc.dram_tensor(
        "x_in", [IMGS, TILE_P, TILE_F], mybir.dt.float32, kind="ExternalInput"
    )
    m_in = nc.dram_tensor("m_in", [TILE_P, TILE_F], mybir.dt.float32, kind="ExternalInput")
    out = nc.dram_tensor(
        "out", [IMGS, TILE_P, TILE_F], mybir.dt.float32, kind="ExternalOutput"
    )

    with tile.TileContext(nc) as tc:
        with (
            tc.tile_pool(name="maskp", bufs=1) as mp,
            tc.tile_pool(name="data", bufs=bufs) as dp,
        ):
            mt = mp.tile([TILE_P, TILE_F], mybir.dt.float32)
            nc.sync.dma_start(mt[:], m_in[:])
            for t in range(IMGS):
                d = dp.tile([TILE_P, TILE_F], mybir.dt.float32)
                nc.sync.dma_start(d[:], x_in[t, :, :])
                nc.vector.tensor_mul(d[:], d[:], mt[:])
                nc.scalar.dma_start(out[t, :, :], d[:])
    nc.finalize()
    return nc


def _get_program():
    if "nc" not in _cache:
        _cache["nc"] = _build_program()
    return _cache["nc"]


def _compute_mask(cv: float) -> np.ndarray:
    """Replicates reference's mask math in numpy f32: [N, N]."""
    template = np.linspace(1.0 - MAX_SIZE, 0.0, MAX_SIZE, dtype=np.float32)
    one_d = np.clip(
        (template + np.float32(cv) * MAX_SIZE) / np.float32(RAMP_SIZE) + np.float32(1.0),
        np.float32(0.0),
        np.float32(1.0),
    ).astype(np.float32)
    one_d = one_d[-(N // 2):]  # [128]
    idx = np.arange(N)
    ring = np.minimum(
        np.minimum(idx[:, None], idx[None, :]),
        np.minimum(N - 1 - idx[:, None], N - 1 - idx[None, :]),
    )  # values in [0, 127] for N=256 — always < N//2, no center special case
    return one_d[ring]


def _run(x, current_val, **spmd_kwargs):
    x = np.ascontiguousarray(np.asarray(x), dtype=np.float32)
    cv = float(np.asarray(current_val).reshape(-1)[0])
    assert x.shape == (B, H, N, N), x.shape

    mask = _compute_mask(cv)  # [256, 256]
    m_t = np.ascontiguousarray(mask.reshape(TILE_P, TILE_F))

    per_core = B // N_CORES
    in_maps = [
        {
            "x_in": x[c * per_core : (c + 1) * per_core].reshape(IMGS, TILE_P, TILE_F),
            "m_in": m_t,
        }
        for c in range(N_CORES)
    ]

    nc = _get_program()
    res = run_bass_kernel_spmd(nc, in_maps, list(range(N_CORES)), **spmd_kwargs)
    out = np.concatenate(
        [r["out"].reshape(per_core, H, N, N) for r in res.results], axis=0
    )
    return out, res


def kernel(x, current_val):
    return _run(x, current_val)[0]


if __name__ == "__main__":
    xs = np.random.randn(B, H, N, N).astype(np.float32)
    cv = np.array([0.1], dtype=np.float32)
    o = kernel(x=xs, current_val=cv)
    expected = xs * _compute_mask(0.1)
    print("self-check max abs diff:", np.abs(o - expected).max())
